# revision 10
# baseline (speedup 1.0000x reference)
"""Trainium2 Bass kernel for MC2RowParallelLinear: Y = X @ W^T + bias.

Full shapes: X [4096, 2, 8192] f32, W [2048, 8192] f32, bias [2048] f32,
Y [4096, 2, 2048] f32.

Strategy (8 NeuronCores): data-parallel over the sequence dim — each core
owns 512 seq rows (1024 flattened [s,b] rows) and computes its Y shard with
the full weight. No collectives needed; the host gathers shards. Inputs are
pre-transposed on the host into k-major layouts so the device does zero
transposes and every DMA is a contiguous ~1 MiB block.

Device kernel (per core): the N dim is processed in two sequential halves.
Within a half, K streams in 8 passes of 8 k-tiles; each PSUM accumulation
group is 8 back-to-back matmuls, and a per-half Y accumulator lives in SBUF
across passes (W read once, X twice, 136 MiB/core total — under the PE
roofline). Matmuls run in float32r (full-rate fp32 on the PE, ~1e-4 max rel
err at K=8192); accumulation is exact fp32 in PSUM/SBUF.
"""

import numpy as np

import concourse.bacc as bacc
import concourse.mybir as mybir
import concourse.tile as tile
from concourse.bass_utils import run_bass_kernel_spmd

S, B, K, N = 4096, 2, 8192, 2048
CORES = 8
SB = S * B           # 8192 flattened rows
SBL = SB // CORES    # 1024 rows per core
P = 128
KT = K // P          # 64 k-tiles
KQ = 8               # k passes per half (Y_acc += per pass)
KTQ = KT // KQ       # 8 k-tiles per pass = one PSUM accumulation group
ST = SBL // P        # 8 sb tiles per core
G = 2                # sb tiles per X block (1 MiB DMA granularity)
STG = ST // G        # 4 X blocks per (core, k-pass)
NH = 2               # n halves, processed sequentially
NHW = N // NH        # 1024
NBW = 512            # n block width (one PSUM bank, 4-byte moving-op max)
NB = NHW // NBW      # 2 n blocks per half
WPR = 2              # W k-rows paired per DMA (1 MiB loads)

MDT = mybir.dt.float32r
F32 = mybir.dt.float32

_cache = {}


def build(reps=1):
    """reps>1 wraps the GEMM body in a hardware loop — timing-only variant."""
    import contextlib

    nc = bacc.Bacc(None, target_bir_lowering=False)
    xt = nc.dram_tensor("xt", [KQ, STG, P, KTQ, G * P], MDT, kind="ExternalInput")
    wt = nc.dram_tensor("wt", [KT // WPR, P, WPR, N], MDT, kind="ExternalInput")
    bias = nc.dram_tensor("bias", [P, N], F32, kind="ExternalInput")
    y = nc.dram_tensor("y", [ST, P, N], F32, kind="ExternalOutput")
    with tile.TileContext(nc) as tc:
        with tc.tile_pool(name="wp", bufs=2 * KTQ // WPR + 1) as wp, \
             tc.tile_pool(name="xp", bufs=3) as xp, \
             tc.tile_pool(name="acc", bufs=1) as accp, \
             tc.tile_pool(name="cst", bufs=1) as cst, \
             tc.tile_pool(name="ps", bufs=8, space="PSUM") as psp:
            bias_sb = cst.tile([P, N], F32, tag="bias")
            nc.sync.dma_start(bias_sb[:], bias[:])
            yaccs = [accp.tile([P, NHW], F32, tag=f"yacc{st}", name=f"yacc{st}")
                     for st in range(ST)]
            loop = tc.For_i(0, reps, 1) if reps > 1 else contextlib.nullcontext()
            with loop:
                _body(nc, wp, xp, psp, xt, wt, y, bias_sb, yaccs)
    nc.compile()
    return nc


def _body(nc, wp, xp, psp, xt, wt, y, bias_sb, yaccs):
    for h in range(NH):
        for kq in range(KQ):
            # W panel for this (half, pass): KTQ k-rows as KTQ/WPR paired
            # 1 MiB loads; pool slack double-buffers the next panel.
            wprs = []
            for pr in range(KTQ // WPR):
                w = wp.tile([P, WPR, NHW], MDT, tag="w", name=f"w_{h}_{kq}_{pr}")
                nc.sync.dma_start(
                    w[:], wt[kq * KTQ // WPR + pr, :, :, h * NHW:(h + 1) * NHW])
                wprs.append(w)
            for stg in range(STG):
                xblk = xp.tile([P, KTQ, G * P], MDT, tag="x",
                               name=f"x_{h}_{kq}_{stg}")
                nc.sync.dma_start(xblk[:], xt[kq, stg])
                for g in range(G):
                    st = stg * G + g
                    for nb in range(NB):
                        ps = psp.tile([P, NBW], F32, tag="ps",
                                      name=f"ps_{h}_{kq}_{st}_{nb}")
                        for ktq in range(KTQ):
                            nc.tensor.matmul(
                                ps[:],
                                xblk[:, ktq, g * P:(g + 1) * P],
                                wprs[ktq // WPR][:, ktq % WPR,
                                                 nb * NBW:(nb + 1) * NBW],
                                start=(ktq == 0), stop=(ktq == KTQ - 1))
                        ysl = yaccs[st][:, nb * NBW:(nb + 1) * NBW]
                        if kq == 0:
                            nc.vector.tensor_add(
                                ysl, ps[:],
                                bias_sb[:, h * NHW + nb * NBW:
                                        h * NHW + (nb + 1) * NBW])
                        else:
                            nc.vector.tensor_add(ysl, ysl, ps[:])
                    if kq == KQ - 1:
                        nc.sync.dma_start(
                            y[st, :, h * NHW:(h + 1) * NHW], yaccs[st][:])


def shard_inputs(input_, weight, bias):
    X = np.ascontiguousarray(np.asarray(input_, np.float32)).reshape(SB, K)
    W = np.ascontiguousarray(np.asarray(weight, np.float32))
    b = np.ascontiguousarray(np.asarray(bias, np.float32))
    # W^T [K, N] with K split (pair, p, j): row k = (pr*WPR + j)*P + p
    WT = np.ascontiguousarray(
        W.T.reshape(KT // WPR, WPR, P, N).transpose(0, 2, 1, 3))
    bias_rep = np.ascontiguousarray(np.broadcast_to(b, (P, N)))
    in_maps = []
    for c in range(CORES):
        Xl = X[c * SBL:(c + 1) * SBL]
        # row = (stg*G + g)*P + sb, col = (kq*KTQ + ktq)*P + p
        #   -> [kq, stg, p, ktq, g*P + sb]
        xt = np.ascontiguousarray(
            Xl.reshape(STG, G, P, KQ, KTQ, P)
            .transpose(3, 0, 5, 4, 1, 2)
            .reshape(KQ, STG, P, KTQ, G * P))
        in_maps.append({"xt": xt, "wt": WT, "bias": bias_rep})
    return in_maps


def kernel(input_, weight, bias):
    if "nc" not in _cache:
        _cache["nc"] = build()
    nc = _cache["nc"]
    in_maps = shard_inputs(input_, weight, bias)
    res = run_bass_kernel_spmd(nc, in_maps, core_ids=list(range(CORES)))
    out = np.concatenate([r["y"].reshape(SBL, N) for r in res.results], axis=0)
    return out.reshape(S, B, N)
